# revision 16
# baseline (speedup 1.0000x reference)
"""Distributed causal attention (RoPE + QK-RMSNorm) for TRN2, 8 NeuronCores.

Problem: B=2, T=2048, C=2048, NH=16 heads of H=128; y = Attn(x) with
 q/k = RMSNorm(RoPE(x @ W{q,k}.T)), causal SDPA, out proj Wo.

Sharding: tensor-parallel over heads x data-parallel over batch.
core i = (b = i//4, g = i%4) owns batch b and heads [4g, 4g+4).
Each core computes its 4 heads end-to-end; the Wo row-partial sums are
combined with an on-device ReduceScatter over each batch group of 4 cores
(chunked over 4 query blocks of 512 tokens so comm overlaps compute), so
core (b, g) emits output tokens qc*512 + g*128 .. +128 for each chunk qc.

Numerics: matmuls in float32r (TF32-like, ~1e-4), P/V in bf16 for the
O = P@V matmuls (full-rate at N=129), softmax without max-subtraction
(QK-RMSNorm bounds scores to +/-sqrt(H)=11.3, exp is safe in f32),
rsqrt via exp(-0.5*ln(x)) to stay inside the exp ACT table set.
"""
import os
import sys

if "/opt/trn_rl_repo" not in sys.path:
    sys.path.insert(0, "/opt/trn_rl_repo")

import numpy as np
import ml_dtypes

import concourse.bass as bass
import concourse.mybir as mybir
import concourse.tile as tile
from concourse import bacc
from concourse.bass_utils import run_bass_kernel_spmd
from concourse.masks import make_identity

B, T, C = 2, 2048, 2048
NH, H = 16, 128
HB = 4           # heads per core
G = 4            # head-groups (= cores per batch)
P = 128
NTT = T // P     # 16 token tiles
QC = 512         # query chunk
NQC = T // QC    # 4 query chunks
TS = QC // P     # 4 token tiles per query chunk
CT = C // P      # 16 contraction tiles
EPS = float(np.finfo(np.float32).eps)

F32 = mybir.dt.float32
F32R = mybir.dt.float32r
BF16 = mybir.dt.bfloat16
AF = mybir.ActivationFunctionType
ALU = mybir.AluOpType

_NC_CACHE = {}


def _rsqrt(nc, out_ap, in_ap, scale, bias, scratch_ap):
    """out = 1/sqrt(in*scale + bias) computed as exp(-0.5*ln(in*scale+bias)).

    Stays in the natural_log_exp ACT table set (no Sqrt-set switch).
    """
    nc.scalar.activation(scratch_ap, in_ap, AF.Ln, bias=bias, scale=scale)
    nc.scalar.activation(out_ap, scratch_ap, AF.Exp, scale=-0.5)


def build_nc(with_rs=True, max_stage=4, nqc=NQC):
    """max_stage: 1=KV phase only, 2=+Q proj, 3=+attention, 4=full."""
    SKIP = set(os.environ.get("KN_SKIP", "").split(","))
    nc = bacc.Bacc("TRN2", target_bir_lowering=False, debug=False,
                   num_devices=8 if with_rs else 1)

    xT = nc.dram_tensor("xT", [C, T], F32R, kind="ExternalInput").ap()
    wq = nc.dram_tensor("wq", [C, HB * H], F32R, kind="ExternalInput").ap()
    wk = nc.dram_tensor("wk", [C, HB * H], F32R, kind="ExternalInput").ap()
    wv = nc.dram_tensor("wv", [C, HB * H], F32R, kind="ExternalInput").ap()
    wo = nc.dram_tensor("wo", [HB * H, C], F32R, kind="ExternalInput").ap()
    cos_e = nc.dram_tensor("cos", [T, H // 2], BF16, kind="ExternalInput").ap()
    sin_e = nc.dram_tensor("sin", [T, H // 2], BF16, kind="ExternalInput").ap()
    mask_e = nc.dram_tensor("mask", [TS, P, QC], BF16, kind="ExternalInput").ap()
    qw_e = nc.dram_tensor("qw", [P, HB * H], F32, kind="ExternalInput").ap()
    kw_e = nc.dram_tensor("kw", [P, HB * H], F32, kind="ExternalInput").ap()
    if with_rs:
        out_e = nc.dram_tensor("out", [NQC, P, C], F32, kind="ExternalOutput").ap()
    else:
        out_e = nc.dram_tensor("out", [NQC, QC, C], F32, kind="ExternalOutput").ap()

    with tile.TileContext(nc) as tc:
        with tc.tile_pool(name="const", bufs=1) as cpool, \
             tc.tile_pool(name="wpool", bufs=2) as wpool, \
             tc.tile_pool(name="big", bufs=1) as bigpool, \
             tc.tile_pool(name="xs", bufs=2) as xpool, \
             tc.tile_pool(name="work", bufs=2) as wk_pool, \
             tc.tile_pool(name="ptile", bufs=3) as ppool, \
             tc.tile_pool(name="obuf", bufs=3) as opool, \
             tc.tile_pool(name="ccdram", bufs=2, space="DRAM") as ccin_pool, \
             tc.tile_pool(name="psA", bufs=2, space="PSUM") as psA, \
             tc.tile_pool(name="psS", bufs=2, space="PSUM") as psS, \
             tc.tile_pool(name="psO", bufs=4, space="PSUM") as psO:

            # ---- constants ----
            ident = cpool.tile([P, P], F32)
            make_identity(nc, ident[:])
            cos_sb = cpool.tile([P, NTT, H // 2], BF16)
            sin_sb = cpool.tile([P, NTT, H // 2], BF16)
            nc.sync.dma_start(cos_sb[:], cos_e.rearrange("(tt p) j -> p tt j", p=P))
            nc.sync.dma_start(sin_sb[:], sin_e.rearrange("(tt p) j -> p tt j", p=P))
            mask_sb = cpool.tile([P, TS, QC], BF16)
            nc.sync.dma_start(mask_sb[:], mask_e.rearrange("d p t -> p d t"))
            qw_sb = cpool.tile([P, HB * H], F32)
            kw_sb = cpool.tile([P, HB * H], F32)
            nc.sync.dma_start(qw_sb[:], qw_e)
            nc.sync.dma_start(kw_sb[:], kw_e)
            epsq_sb = cpool.tile([P, 1], F32)
            epsk_sb = cpool.tile([P, 1], F32)
            nc.vector.memset(epsq_sb[:], float(H) * EPS)
            nc.vector.memset(epsk_sb[:], EPS)

            # ---- persistent big tensors ----
            kT_sb = bigpool.tile([P, HB, NTT, P], F32R)       # [h, hb, kt, tk]
            v_sb = bigpool.tile([P, NTT, HB, H + 1], BF16)    # [tk, kt, hb, h|1]
            nc.vector.memset(v_sb[:, :, :, H:H + 1], 1.0)

            # ---- weights (2 cycling slots of [128, 16, 512]) ----
            wk_sb = wpool.tile([P, CT, HB * H], F32R, tag="w")
            wv_sb = wpool.tile([P, CT, HB * H], F32R, tag="w")
            nc.sync.dma_start(wk_sb[:], wk.rearrange("(ct p) h -> p ct h", p=P))
            nc.sync.dma_start(wv_sb[:], wv.rearrange("(ct p) h -> p ct h", p=P))

            def proj_norm_transpose(tt, x_tile, w_sb, w_bcast, is_q, dst_T, dst_col):
                """Project one 128-token tile for 4 heads, RoPE+RMSNorm it, and
                write the transposed [h, t] tiles into dst_T[:, hb, dst_col]."""
                pp = psA.tile([P, HB, H], F32, tag="proj")
                for ct in range(CT):
                    nc.tensor.matmul(
                        pp[:].rearrange("p hb h -> p (hb h)"),
                        x_tile[:, ct, :], w_sb[:, ct, :],
                        start=(ct == 0), stop=(ct == CT - 1))
                # rope: pairs (j, j+64); cos/sin [128t, 64]
                qn = wk_pool.tile([P, HB, H], F32, tag="qn")
                if "rope" in SKIP:
                    nc.vector.tensor_copy(qn[:], pp[:])
                r1 = wk_pool.tile([P, HB, H // 2], F32, tag="r1")
                r2 = wk_pool.tile([P, HB, H // 2], F32, tag="r2")
                if "rope" not in SKIP:
                    cos_b = cos_sb[:, tt, :].unsqueeze(1).broadcast_to([P, HB, H // 2])
                    sin_b = sin_sb[:, tt, :].unsqueeze(1).broadcast_to([P, HB, H // 2])
                    x1 = pp[:, :, 0:H // 2]
                    x2 = pp[:, :, H // 2:H]
                    nc.vector.tensor_mul(r1[:], x1, cos_b)
                    nc.vector.tensor_mul(r2[:], x2, sin_b)
                    nc.vector.tensor_sub(qn[:, :, 0:H // 2], r1[:], r2[:])
                    nc.vector.tensor_mul(r1[:], x1, sin_b)
                    nc.vector.tensor_mul(r2[:], x2, cos_b)
                    nc.vector.tensor_add(qn[:, :, H // 2:H], r1[:], r2[:])
                # mean-square of the roped value: square then reduce per head
                ms = wk_pool.tile([P, HB], F32, tag="ms")
                scr = wk_pool.tile([P, HB, H], F32, tag="scr")
                if "ms" in SKIP:
                    nc.vector.memset(ms[:], 1.0)
                else:
                    nc.vector.tensor_mul(scr[:], qn[:], qn[:])
                    nc.vector.tensor_reduce(
                        out=ms[:], in_=scr[:], op=ALU.add,
                        axis=mybir.AxisListType.X)
                # rs = 1/sqrt(ms*scale + bias); q also folds in 1/sqrt(H)
                rs = wk_pool.tile([P, HB], F32, tag="rs")
                lnscr = wk_pool.tile([P, HB], F32, tag="lnscr")
                if "norm" not in SKIP:
                    if is_q:
                        _rsqrt(nc, rs[:], ms[:], 1.0, epsq_sb[:], lnscr[:])
                    else:
                        _rsqrt(nc, rs[:], ms[:], 1.0 / H, epsk_sb[:], lnscr[:])
                    for hb in range(HB):
                        nc.vector.tensor_scalar_mul(
                            qn[:, hb, :], qn[:, hb, :], rs[:, hb:hb + 1])
                    nc.vector.tensor_mul(
                        qn[:].rearrange("p hb h -> p (hb h)"),
                        qn[:].rearrange("p hb h -> p (hb h)"), w_bcast[:])
                if "tr" in SKIP:
                    return
                # transpose each [128t, 128h] head tile -> [h, t]
                tp = psS.tile([P, HB, P], F32, tag="tp")
                for hb in range(HB):
                    nc.tensor.transpose(tp[:, hb, :], qn[:, hb, :], ident[:])
                nc.vector.tensor_copy(dst_T[:, :, dst_col, :], tp[:])

            # ================= phase 0: K, V for all tokens =================
            ntt0 = int(os.environ.get("KN_NTT", NTT))
            for tt in range(ntt0):
                x_tile = xpool.tile([P, CT, P], F32R, tag="xs")
                nc.sync.dma_start(
                    x_tile[:],
                    xT[:, tt * P:(tt + 1) * P].rearrange("(ct p) t -> p ct t", p=P))
                if "kproj" not in SKIP:
                    proj_norm_transpose(tt, x_tile, wk_sb, kw_sb, False, kT_sb, tt)
                if "v" in SKIP:
                    continue
                vp = psA.tile([P, HB, H], F32, tag="proj")
                for ct in range(CT):
                    nc.tensor.matmul(
                        vp[:].rearrange("p hb h -> p (hb h)"),
                        x_tile[:, ct, :], wv_sb[:, ct, :],
                        start=(ct == 0), stop=(ct == CT - 1))
                nc.scalar.copy(v_sb[:, tt, :, 0:H], vp[:])

            # ---- phase 1 weights (reuse the two w slots) ----
            wq_sb = wpool.tile([P, CT, HB * H], F32R, tag="w")
            wo_sb = wpool.tile([P, HB, C], F32R, tag="w")
            nc.sync.dma_start(wq_sb[:], wq.rearrange("(ct p) h -> p ct h", p=P))
            nc.sync.dma_start(wo_sb[:], wo.rearrange("(fb p) c -> p fb c", p=P))

            # ================= phase 1: per query chunk =================
            qT_sb = bigpool.tile([P, HB, TS, P], F32R)    # [h, hb, ts, tq]
            a_sb = bigpool.tile([P, TS, HB * H], F32)     # [tq, ts, f]
            aT_sb = bigpool.tile([P, HB, QC], F32R)       # [f, fb, tq]

            for qc in range(nqc if max_stage >= 2 else 0):
                # -- Q projection + norm + transpose --
                for ts in range(TS):
                    tt = qc * TS + ts
                    x_tile = xpool.tile([P, CT, P], F32R, tag="xs")
                    nc.sync.dma_start(
                        x_tile[:],
                        xT[:, tt * P:(tt + 1) * P].rearrange(
                            "(ct p) t -> p ct t", p=P))
                    proj_norm_transpose(tt, x_tile, wq_sb, qw_sb, True, qT_sb, ts)

                # -- attention per head --
                nkt = (qc + 1) * TS
                for hb in range(HB if max_stage >= 3 else 0):
                    o_ps = [psO.tile([P, H + 1], F32, tag="o",
                                     name=f"o_{qc}_{hb}_{i}") for i in range(TS)]
                    for kt in range(nkt):
                        s_ps = psS.tile([P, QC], F32, tag="tp")
                        nc.tensor.matmul(
                            s_ps[:], kT_sb[:, hb, kt, :],
                            qT_sb[:, hb, :, :].rearrange("p ts t -> p (ts t)"),
                            start=True, stop=True)
                        p_sb = ppool.tile([P, QC], BF16, tag="p")
                        nc.scalar.activation(p_sb[:], s_ps[:], AF.Exp)
                        d = kt - qc * TS
                        if d >= 0:
                            nc.vector.tensor_mul(
                                p_sb[:], p_sb[:], mask_sb[:, d, :])
                        for ts in range(TS):
                            nc.tensor.matmul(
                                o_ps[ts][:], p_sb[:, ts * P:(ts + 1) * P],
                                v_sb[:, kt, hb, :],
                                start=(kt == 0), stop=(kt == nkt - 1))
                    for ts in range(TS):
                        den = wk_pool.tile([P, 1], F32, tag="den")
                        nc.vector.reciprocal(den[:], o_ps[ts][:, H:H + 1])
                        nc.vector.tensor_scalar_mul(
                            a_sb[:, ts, hb * H:(hb + 1) * H],
                            o_ps[ts][:, 0:H], den[:])

                if max_stage < 4:
                    continue
                # -- A^T --
                for fb in range(HB):
                    tp = psA.tile([P, TS, P], F32, tag="proj")
                    for ts in range(TS):
                        nc.tensor.transpose(
                            tp[:, ts, :], a_sb[:, ts, fb * P:(fb + 1) * P],
                            ident[:])
                    nc.vector.tensor_copy(
                        aT_sb[:, fb, :], tp[:].rearrange("p ts t -> p (ts t)"))

                # -- Wo partial + ReduceScatter --
                bounce = ccin_pool.tile([QC, C], F32, tag="bounce")
                for ts in range(TS):
                    for cc in range(C // QC):
                        wo_ps = psS.tile([P, QC], F32, tag="tp")
                        for fb in range(HB):
                            nc.tensor.matmul(
                                wo_ps[:], aT_sb[:, fb, ts * P:(ts + 1) * P],
                                wo_sb[:, fb, cc * QC:(cc + 1) * QC],
                                start=(fb == 0), stop=(fb == HB - 1))
                        ob = opool.tile([P, QC], F32, tag="ob")
                        nc.scalar.copy(ob[:], wo_ps[:])
                        if with_rs:
                            nc.sync.dma_start(
                                bounce[ts * P:(ts + 1) * P,
                                       cc * QC:(cc + 1) * QC],
                                ob[:])
                        else:
                            nc.sync.dma_start(
                                out_e[qc, ts * P:(ts + 1) * P,
                                      cc * QC:(cc + 1) * QC],
                                ob[:])
                if with_rs:
                    red = ccin_pool.tile([P, C], F32, tag="red",
                                         name=f"red{qc}")
                    nc.gpsimd.collective_compute(
                        "ReduceScatter",
                        ALU.add,
                        ins=[bounce[:].opt()],
                        outs=[red[:].opt()],
                        replica_groups=[[0, 1, 2, 3], [4, 5, 6, 7]],
                    )
                    nc.sync.dma_start(out_e[qc], red[:])

    nc.compile()
    return nc


def _get_nc():
    if "nc" not in _NC_CACHE:
        _NC_CACHE["nc"] = build_nc()
    return _NC_CACHE["nc"]


def make_in_maps(x, sin, cos, Wq, Wk, Wv, Wo, q_norm_w, k_norm_w):
    cos_b = np.ascontiguousarray(cos).astype(ml_dtypes.bfloat16)
    sin_b = np.ascontiguousarray(sin).astype(ml_dtypes.bfloat16)
    # mask[d, tk, tq] = 1 iff d*128 + tk <= tq   (S^T tile: [tk, tq])
    d_idx = np.arange(TS)[:, None, None]
    tk_idx = np.arange(P)[None, :, None]
    tq_idx = np.arange(QC)[None, None, :]
    mask = ((d_idx * P + tk_idx) <= tq_idx).astype(ml_dtypes.bfloat16)
    qw = np.tile(np.asarray(q_norm_w, np.float32)[None, :], (P, HB))
    kw = np.tile(np.asarray(k_norm_w, np.float32)[None, :], (P, HB))
    in_maps = []
    for i in range(8):
        b, g = divmod(i, G)
        sl = slice(g * HB * H, (g + 1) * HB * H)
        in_maps.append({
            "xT": np.ascontiguousarray(np.asarray(x[b], np.float32).T),
            "wq": np.ascontiguousarray(np.asarray(Wq, np.float32)[sl, :].T),
            "wk": np.ascontiguousarray(np.asarray(Wk, np.float32)[sl, :].T),
            "wv": np.ascontiguousarray(np.asarray(Wv, np.float32)[sl, :].T),
            "wo": np.ascontiguousarray(np.asarray(Wo, np.float32)[:, sl].T),
            "cos": cos_b, "sin": sin_b, "mask": mask, "qw": qw, "kw": kw,
        })
    return in_maps


def assemble_output(results):
    out = np.empty((B, T, C), np.float32)
    for i in range(8):
        b, g = divmod(i, G)
        r = results[i]["out"]  # [NQC, P, C]
        for qc in range(NQC):
            t0 = qc * QC + g * P
            out[b, t0:t0 + P, :] = r[qc]
    return out


def kernel(x, sin, cos, Wq, Wk, Wv, Wo, q_norm_w, k_norm_w):
    nc = _get_nc()
    in_maps = make_in_maps(x, sin, cos, Wq, Wk, Wv, Wo, q_norm_w, k_norm_w)
    res = run_bass_kernel_spmd(nc, in_maps, core_ids=list(range(8)))
    return assemble_output(res.results)
